# revision 22
# baseline (speedup 1.0000x reference)
"""Trainium2 Bass kernel for nn_MultiHeadAttention (B=16 heads, S=2048, D=1024, DH=64).

Sharding: 2 heads per core across 8 cores (head-parallel), NO collectives:
each core computes its 2 heads' slice of the final W_O GEMM (contract over
the 128 local head-dims) into a full-shape [S, D] bf16 partial; the host
sums the 8 partials and adds bo. Data that must cross cores is identical
either way; this removes collective latency and the cc readback.

Per-core program (all bf16 matmuls, fp32 PSUM):
  - projections: c-outer accumulation with zero-padded stationaries
    wz0=[W|0], wz1=[0|W] so both heads' halves accumulate into one full
    [128,512] PSUM tile (junk rows are exact zeros) and consecutive
    matmuls share identical ldweights (deduped post-pass).
  - scores: stationary kT2[:,j*128:+128] [128,128] (both heads stacked),
    moving = zero-padded qz0/qz1 so the cross-head contract terms vanish;
    one ldweights per (sqq,j) pair after dedup. exp on ACT engine
    ([128,1024] per group), denominator accumulated on DVE.
  - AV: stationary vaz0=[va_h0|0], vaz1=[0|va_h1] accumulate both heads
    into one [128,512] PSUM tile; j0:7 partial stashed to SBUF (bf16) so
    only 2 PSUM ot-tiles are ever live.
  - y: per sqq, 4 stationary cc-chunks [128,128] x moving Wo2 [128,1024]
    -> y s-chunk [128,1024] fp32 -> bf16 -> DMA out. 16 chunks total.
  - startup: S-half input DMAs (k,q,v interleaved by deadline) feed a
    phase schedule that keeps PE busy from ~6us on.
"""
import sys, os
sys.path.insert(0, '/opt/trn_rl_repo')
import numpy as np

B = 16        # total heads
S = 2048
D = 1024
DH = 64
N_CORES = 8
HPC = B // N_CORES          # heads per core = 2
EXN = int(os.environ.get("KEXN", "24"))   # ex-tile pool depth

_runner = None


def _split_excess_waits(nc, mybir):
    """walrus in this env supports only ONE sync-wait command per instruction;
    hoist extra waits onto preceding single-wait NOPs on the same engine."""
    for f in nc.m.functions:
        for blk in f.blocks:
            new_list = []
            changed = False
            for ins in blk.instructions:
                si = ins.sync_info
                if si is not None and si.on_wait and len(si.on_wait) > 1:
                    waits = list(si.on_wait)
                    extra, keep = waits[:-1], waits[-1:]
                    for ci, w in enumerate(extra):
                        nop = mybir.InstNoOp(name=f"{ins.name}_wsplit_{ci}", ins=[], outs=[])
                        nop.engine = ins.engine
                        nop.sync_info = mybir.SyncInfo(on_wait=[w], on_update=[])
                        new_list.append(nop)
                    ins.sync_info = mybir.SyncInfo(on_wait=keep, on_update=list(si.on_update))
                    changed = True
                new_list.append(ins)
            if changed:
                blk.instructions = new_list


def _dedup_ldweights(nc, mybir):
    """Remove an InstLdweights that re-loads exactly what the previous
    InstLdweights on PE already loaded (same AP/tile/flags). The legalizer
    emits one ldweights per matmult even when consecutive matmuls share the
    stationary; the PE array retains it, so the duplicate is a pure re-load.
    Only drop duplicates that carry no waits; sem updates are moved to the
    next instruction."""

    def sig(ins):
        pap = ins.ins[0]
        return (pap.offset, tuple(tuple(x) for x in pap.ap), str(pap.dtype),
                tuple(ins.tile_position or ()), tuple(ins.tile_size or ()),
                bool(ins.is_transpose), ins.perf_mode)

    removed = 0
    for f in nc.m.functions:
        for blk in f.blocks:
            last_sig = None
            keep = []
            pending_updates = []
            for ins in blk.instructions:
                if isinstance(ins, mybir.InstLdweights):
                    s = sig(ins)
                    si = ins.sync_info
                    if (s == last_sig and not ins.is_transpose
                            and (si is None or not si.on_wait)):
                        if si is not None and si.on_update:
                            pending_updates.extend(si.on_update)
                        removed += 1
                        continue
                    last_sig = s
                elif getattr(ins, 'engine', None) == mybir.EngineType.PE and \
                        not isinstance(ins, mybir.InstMatmult):
                    # any other PE instruction invalidates the array state
                    last_sig = None
                if pending_updates:
                    si = ins.sync_info
                    w = list(si.on_wait) if si else []
                    u = (list(si.on_update) if si else []) + pending_updates
                    ins.sync_info = mybir.SyncInfo(on_wait=w, on_update=u)
                    pending_updates = []
                keep.append(ins)
            assert not pending_updates
            blk.instructions = keep
    return removed


def build_nc(repeat=1, with_tail=True):
    """Build the per-core Bass program. repeat>1 wraps the body in a hardware
    loop (bench mode). with_tail kept for test.py compat (no tail anymore)."""
    import concourse.bass as bass
    import concourse.mybir as mybir
    import concourse.tile as tile
    from concourse.masks import make_identity

    F32 = mybir.dt.float32
    BF16 = mybir.dt.bfloat16
    AF = mybir.ActivationFunctionType

    nc = bass.Bass()

    qt_ext = nc.declare_dram_parameter("qt", [HPC, 128, 8, S], BF16, isOutput=False)
    kt_ext = nc.declare_dram_parameter("kt", [HPC, 128, 8, S], BF16, isOutput=False)
    vt_ext = nc.declare_dram_parameter("vt", [HPC, 128, 8, S], BF16, isOutput=False)
    wz_ext = {nm: nc.declare_dram_parameter(f"w{nm}_t", [128, 8, DH], BF16, isOutput=False)
              for nm in ("q", "k", "v")}
    b_ext = {nm: nc.declare_dram_parameter(f"b{nm}2", [128, 1], F32, isOutput=False)
             for nm in ("q", "k", "v")}
    wo_ext = nc.declare_dram_parameter("wo2", [128, D], BF16, isOutput=False)
    y_ext = nc.declare_dram_parameter("y", [16, 128, D], BF16, isOutput=True)

    with tile.TileContext(nc) as tc:
        with (
            tc.tile_pool(name="consts", bufs=1) as consts,
            nc.allow_low_precision(reason="bf16 matmuls by design"),
        ):
            # ---- constants ----
            ident_f32 = consts.tile([128, 128], F32)
            make_identity(nc, ident_f32)
            ident_bf = consts.tile([128, 128], BF16)
            nc.vector.tensor_copy(ident_bf[:], ident_f32[:])
            ones128 = consts.tile([128, 1], BF16)
            nc.vector.memset(ones128, 1.0)
            onesz0 = consts.tile([1, 128], BF16)
            nc.vector.memset(onesz0[:, 0:64], 1.0)
            nc.vector.memset(onesz0[:, 64:128], 0.0)
            onesz1 = consts.tile([1, 128], BF16)
            nc.vector.memset(onesz1[:, 0:64], 0.0)
            nc.vector.memset(onesz1[:, 64:128], 1.0)

            # weight/bias DMAs are issued lazily inside the schedule so input
            # DMAs aren't delayed behind them; tiles declared here.
            biases = {}
            wz = {}
            for nm in ("q", "k", "v"):
                biases[nm] = consts.tile([128, 1], F32, tag=f"b{nm}", name=f"b{nm}")
                wz[nm] = consts.tile([128, 8, DH], BF16, tag=f"w{nm}", name=f"w{nm}")
            wo_sb = consts.tile([128, D], BF16)

            def dma_w(nm):
                # issue on Pool queue: keeps the SP queue a pure input stream
                nc.gpsimd.dma_start(out=biases[nm][:], in_=b_ext[nm][:])
                nc.gpsimd.dma_start(out=wz[nm][:], in_=wz_ext[nm][:])

            # ---- persistent per-iteration tensors ----
            kT2 = consts.tile([128, S], BF16)
            vT2 = consts.tile([128, S], BF16)
            qT2 = consts.tile([128, S], BF16)   # rows 0:64 = h0, 64:128 = h1
            va = consts.tile([128, 16, 128], BF16)  # [k, j, h-stack]
            cc_sbuf = consts.tile([128, S], BF16)
            dma_w("k")
            dma_w("q")
            dma_w("v")
            nc.gpsimd.dma_start(out=wo_sb[:], in_=wo_ext[:])

            with (
                tc.tile_pool(name="inp", bufs=7) as in_pool,
                tc.tile_pool(name="expp", bufs=EXN) as ex_pool,
                tc.tile_pool(name="denp", bufs=8) as den_pool,
                tc.tile_pool(name="accp", bufs=4) as acc_pool,
                tc.tile_pool(name="smal", bufs=4) as small_pool,
                tc.tile_pool(name="ysbp", bufs=2) as y_pool,
                tc.tile_pool(name="sc_ps", bufs=2, space="PSUM") as sc_ps,
                tc.tile_pool(name="ot_ps", bufs=2, space="PSUM") as ot_ps,
                tc.tile_pool(name="pj_ps", bufs=2, space="PSUM") as pj_ps,
            ):
                def compute_body(_iv=None):
                    # ---- input DMA: quarter tiles [128, 8, 512] on demand ----
                    ins_sb = {}

                    exts = {"k": kt_ext, "q": qt_ext, "v": vt_ext}

                    def dma_in(tname, qi, splits=1):
                        ext = exts[tname]
                        ts = []
                        for h in range(2):
                            t = in_pool.tile([128, 8, 512], BF16, tag="in",
                                             name=f"in_{tname}{h}{qi}")
                            ins_sb[(tname, h, qi)] = t
                            ts.append(t)
                        w = 512 // splits
                        for sp in range(splits):
                            for h in range(2):
                                nc.sync.dma_start(
                                    out=ts[h][:, :, sp * w:(sp + 1) * w],
                                    in_=ext[h, :, :, qi * 512 + sp * w: qi * 512 + (sp + 1) * w])

                    # ---- projection: one quarter (S-cols qi*512:+512) ----
                    def proj_q(nm, qi, splits=1):
                        ta = ins_sb.pop((nm, 0, qi))
                        tb = ins_sb.pop((nm, 1, qi))
                        wt = wz[nm]
                        ps = pj_ps.tile([128, 512], F32, tag="pp", name=f"pj{nm}{qi}")
                        w = 512 // splits
                        for sp in range(splits):
                            sub = slice(sp * w, (sp + 1) * w)
                            for c in range(8):
                                nc.tensor.matmul(ps[0:64, sub], wt[:, c, :], ta[:, c, sub],
                                                 start=(c == 0), stop=(c == 7),
                                                 skip_group_check=True)
                            for c in range(8):
                                nc.tensor.matmul(ps[64:128, sub], wt[:, c, :], tb[:, c, sub],
                                                 start=(c == 0), stop=(c == 7),
                                                 skip_group_check=True)
                        blk = slice(qi * 512, (qi + 1) * 512)
                        dst = {"k": kT2, "v": vT2, "q": qT2}[nm]
                        nc.scalar.activation(dst[:, blk], ps[:],
                                             AF.Identity, bias=biases[nm])

                    # ---- attention pieces ----
                    ex_tiles, ots, accs, dens, bcs = {}, {}, {}, {}, {}

                    def sc_group(s, j):
                        """scores + exp + den for (sqq s, key-chunk j)."""
                        s0 = s * 512
                        jsl = slice(j * 128, (j + 1) * 128)
                        sc = sc_ps.tile([128, 1024], F32, tag="sc", name=f"sc{s}_{j}")
                        nc.tensor.matmul(sc[:, 0:512], kT2[0:64, jsl],
                                         qT2[0:64, s0:s0 + 512], start=True, stop=True)
                        nc.tensor.matmul(sc[:, 512:1024], kT2[64:128, jsl],
                                         qT2[64:128, s0:s0 + 512], start=True, stop=True)
                        ex = ex_pool.tile([128, 1024], BF16, tag="ex", name=f"ex{s}_{j}")
                        nc.scalar.activation(ex[:], sc[:], AF.Exp)
                        # denominator: even j accumulate on DVE, odd j on Pool
                        eng = nc.vector if j % 2 == 0 else nc.gpsimd
                        half = j % 2
                        if j < 2:
                            den = den_pool.tile([128, 1024], BF16, tag="dn",
                                                name=f"den{s}_{half}")
                            eng.tensor_copy(den[:], ex[:])
                            dens[(s, half)] = den
                        else:
                            d = dens[(s, half)]
                            eng.tensor_add(d[:], d[:], ex[:])
                        ex_tiles[(s, j)] = ex

                    def tp_j(j):
                        """transpose v chunk j for both heads + scatter to vaz."""
                        jsl = slice(j * 128, (j + 1) * 128)
                        tp = ot_ps.tile([128, 128], BF16, tag="ot", name=f"tp{j}")
                        nc.tensor.transpose(tp[:], vT2[:, jsl], ident_bf[:])
                        nc.vector.tensor_copy(va[:, j, :], tp[:])

                    def av_q(s, jq):
                        """AV over key-quarter jq (4 j's) + fold into SBUF acc."""
                        j0, j1 = jq * 4, jq * 4 + 3
                        ot = ot_ps.tile([128, 512], F32, tag="ot", name=f"ot{s}_{jq}")
                        for j in range(j0, j1 + 1):
                            ex = ex_tiles.pop((s, j))
                            nc.tensor.matmul(ot[0:64, :], va[:, j, 0:64], ex[:, 0:512],
                                             start=(j == j0), stop=(j == j1),
                                             skip_group_check=True)
                            nc.tensor.matmul(ot[64:128, :], va[:, j, 64:128],
                                             ex[:, 512:1024],
                                             start=(j == j0), stop=(j == j1),
                                             skip_group_check=True)
                        if jq == 0:
                            acc = acc_pool.tile([128, 512], BF16, tag="acc", name=f"acc{s}")
                            nc.vector.tensor_copy(acc[:], ot[:])
                            accs[s] = acc
                        else:
                            nc.vector.tensor_add(accs[s][:], accs[s][:], ot[:])

                    def norm_a(s):
                        """den -> 1/den broadcast tile (as soon as den ready)."""
                        den_e = dens.pop((s, 0))
                        den_o = dens.pop((s, 1))
                        red = sc_ps.tile([128, 1024], F32, tag="sc", name=f"red{s}")
                        for half in (0, 1):
                            cs = slice(half * 512, (half + 1) * 512)
                            nc.tensor.matmul(red[0:1, cs], ones128[:], den_e[:, cs],
                                             start=True, stop=False,
                                             skip_group_check=True)
                            nc.tensor.matmul(red[0:1, cs], ones128[:], den_o[:, cs],
                                             start=False, stop=True,
                                             skip_group_check=True)
                        rc2 = small_pool.tile([1, 1024], BF16, tag="rc", name=f"rc{s}")
                        nc.vector.tensor_copy(rc2[:], red[0:1, :])
                        bc = sc_ps.tile([128, 1024], F32, tag="sc", name=f"bc{s}")
                        nc.tensor.matmul(bc[:, 0:512], onesz0[:], rc2[:, 0:512],
                                         start=True, stop=False, skip_group_check=True)
                        nc.tensor.matmul(bc[:, 0:512], onesz1[:], rc2[:, 512:1024],
                                         start=False, stop=True, skip_group_check=True)
                        bc_sb = small_pool.tile([128, 512], BF16, tag="bcsb", name=f"bcsb{s}")
                        nc.vector.reciprocal(bc_sb[:], bc[:, 0:512])
                        bcs[s] = bc_sb

                    def norm_b(s):
                        s0 = s * 512
                        nc.vector.tensor_mul(cc_sbuf[:, s0:s0 + 512],
                                             accs.pop(s)[:], bcs.pop(s)[:])

                    def y_chunk(s, i):
                        s0 = s * 512
                        chunk = cc_sbuf[:, s0 + i * 128: s0 + (i + 1) * 128]
                        ya = pj_ps.tile([128, 512], F32, tag="pp", name=f"ya{s}{i}")
                        yb = pj_ps.tile([128, 512], F32, tag="pp", name=f"yb{s}{i}")
                        nc.tensor.matmul(ya[:], chunk, wo_sb[:, 0:512],
                                         start=True, stop=True)
                        nc.tensor.matmul(yb[:], chunk, wo_sb[:, 512:1024],
                                         start=True, stop=True)
                        ysb = y_pool.tile([128, 1024], BF16, tag="ysb", name=f"y{s}{i}")
                        nc.scalar.activation(ysb[:, 0:512], ya[:], AF.Copy)
                        nc.vector.tensor_copy(ysb[:, 512:1024], yb[:])
                        # SWDGE (gpsimd) DMAs fail walrus codegen inside a
                        # hardware loop; use them only in the single-shot build
                        yeng = nc.gpsimd if repeat == 1 else nc.sync
                        yeng.dma_start(out=y_ext[s * 4 + i, :, :], in_=ysb[:])

                    # ================= SCHEDULE =================
                    # The SP queue is a pure input stream in deadline order
                    # (consts went out on the Pool queue before the body).
                    dma_in("k", 0, splits=2)
                    dma_in("q", 0, splits=2)
                    dma_in("k", 1)
                    dma_in("q", 1)
                    dma_in("v", 0)
                    dma_in("k", 2)
                    dma_in("k", 3)
                    dma_in("v", 1)
                    dma_in("q", 2)
                    dma_in("v", 2)
                    dma_in("v", 3)
                    dma_in("q", 3)

                    proj_q("k", 0, splits=2)    # first k bytes land ~@4.5
                    proj_q("q", 0, splits=2)
                    for j in range(0, 4):       # kT j0:3, qz s0 ready ~@15
                        sc_group(0, j)
                    proj_q("k", 1)              # k q1 ~@21
                    for j in range(4, 8):
                        sc_group(0, j)
                    proj_q("q", 1)              # q q1 ~@27
                    for j in range(0, 8):
                        sc_group(1, j)
                    proj_q("v", 0)              # v q0 ~@35
                    for j in range(0, 4):
                        tp_j(j)
                    av_q(0, 0)
                    av_q(1, 0)
                    proj_q("k", 2)              # k q2 ~@41
                    for j in range(8, 12):
                        sc_group(0, j)
                        sc_group(1, j)
                    proj_q("k", 3)              # k q3 ~@47
                    for j in range(12, 16):
                        sc_group(0, j)
                        sc_group(1, j)
                    norm_a(0)
                    norm_a(1)
                    proj_q("v", 1)              # v q1 ~@53
                    for j in range(4, 8):
                        tp_j(j)
                    av_q(0, 1)
                    av_q(1, 1)
                    proj_q("q", 2)              # q q2 ~@59
                    for j in range(0, 8):       # s2 with JIT AV
                        sc_group(2, j)
                        if j == 5:
                            av_q(2, 0)
                    proj_q("v", 2)              # v q2 ~@65
                    for j in range(8, 12):
                        tp_j(j)
                    av_q(0, 2)
                    av_q(1, 2)
                    av_q(2, 1)
                    for j in range(8, 16):      # s2 tail scores, JIT AV
                        sc_group(2, j)
                        if j == 13:
                            av_q(2, 2)
                    norm_a(2)
                    proj_q("q", 3)              # q q3 ~@76
                    proj_q("v", 3)              # v q3 ~@70
                    for j in range(12, 16):
                        tp_j(j)
                    av_q(0, 3)
                    norm_b(0)
                    av_q(1, 3)
                    norm_b(1)
                    av_q(2, 3)
                    norm_b(2)
                    # s0..s2 output GEMMs interleave with s3's attention tail
                    yq = [(s, i) for i in range(4) for s in range(3)]
                    for j in range(0, 16):      # s3 JIT: sc -> AV lag
                        sc_group(3, j)
                        if j == 5:
                            av_q(3, 0)
                        elif j == 9:
                            av_q(3, 1)
                        elif j == 13:
                            av_q(3, 2)
                        elif yq:
                            y_chunk(*yq.pop(0))
                    norm_a(3)
                    av_q(3, 3)
                    while yq:
                        y_chunk(*yq.pop(0))
                    norm_b(3)
                    for i in range(4):
                        y_chunk(3, i)


# revision 23
# speedup vs baseline: 1.5181x; 1.5181x over previous
"""Trainium2 Bass kernel for nn_MultiHeadAttention (B=16 heads, S=2048, D=1024, DH=64).

Sharding: 2 heads per core across 8 cores (head-parallel), NO collectives:
each core computes its 2 heads' slice of the final W_O GEMM (contract over
the 128 local head-dims) into a full-shape [S, D] bf16 partial; the host
sums the 8 partials and adds bo. Data that must cross cores is identical
either way; this removes collective latency and the cc readback.

Per-core program (all bf16 matmuls, fp32 PSUM):
  - projections: c-outer accumulation with zero-padded stationaries
    wz0=[W|0], wz1=[0|W] so both heads' halves accumulate into one full
    [128,512] PSUM tile (junk rows are exact zeros) and consecutive
    matmuls share identical ldweights (deduped post-pass).
  - scores: stationary kT2[:,j*128:+128] [128,128] (both heads stacked),
    moving = zero-padded qz0/qz1 so the cross-head contract terms vanish;
    one ldweights per (sqq,j) pair after dedup. exp on ACT engine
    ([128,1024] per group), denominator accumulated on DVE.
  - AV: stationary vaz0=[va_h0|0], vaz1=[0|va_h1] accumulate both heads
    into one [128,512] PSUM tile; j0:7 partial stashed to SBUF (bf16) so
    only 2 PSUM ot-tiles are ever live.
  - y: per sqq, 4 stationary cc-chunks [128,128] x moving Wo2 [128,1024]
    -> y s-chunk [128,1024] fp32 -> bf16 -> DMA out. 16 chunks total.
  - startup: S-half input DMAs (k,q,v interleaved by deadline) feed a
    phase schedule that keeps PE busy from ~6us on.
"""
import sys, os
sys.path.insert(0, '/opt/trn_rl_repo')
import numpy as np

B = 16        # total heads
S = 2048
D = 1024
DH = 64
N_CORES = 8
HPC = B // N_CORES          # heads per core = 2
EXN = int(os.environ.get("KEXN", "24"))   # ex-tile pool depth

_runner = None


def _split_excess_waits(nc, mybir):
    """walrus in this env supports only ONE sync-wait command per instruction;
    hoist extra waits onto preceding single-wait NOPs on the same engine."""
    for f in nc.m.functions:
        for blk in f.blocks:
            new_list = []
            changed = False
            for ins in blk.instructions:
                si = ins.sync_info
                if si is not None and si.on_wait and len(si.on_wait) > 1:
                    waits = list(si.on_wait)
                    extra, keep = waits[:-1], waits[-1:]
                    for ci, w in enumerate(extra):
                        nop = mybir.InstNoOp(name=f"{ins.name}_wsplit_{ci}", ins=[], outs=[])
                        nop.engine = ins.engine
                        nop.sync_info = mybir.SyncInfo(on_wait=[w], on_update=[])
                        new_list.append(nop)
                    ins.sync_info = mybir.SyncInfo(on_wait=keep, on_update=list(si.on_update))
                    changed = True
                new_list.append(ins)
            if changed:
                blk.instructions = new_list


def _dedup_ldweights(nc, mybir):
    """Remove an InstLdweights that re-loads exactly what the previous
    InstLdweights on PE already loaded (same AP/tile/flags). The legalizer
    emits one ldweights per matmult even when consecutive matmuls share the
    stationary; the PE array retains it, so the duplicate is a pure re-load.
    Only drop duplicates that carry no waits; sem updates are moved to the
    next instruction."""

    def sig(ins):
        pap = ins.ins[0]
        return (pap.offset, tuple(tuple(x) for x in pap.ap), str(pap.dtype),
                tuple(ins.tile_position or ()), tuple(ins.tile_size or ()),
                bool(ins.is_transpose), ins.perf_mode)

    removed = 0
    for f in nc.m.functions:
        for blk in f.blocks:
            last_sig = None
            keep = []
            pending_updates = []
            for ins in blk.instructions:
                if isinstance(ins, mybir.InstLdweights):
                    s = sig(ins)
                    si = ins.sync_info
                    if (s == last_sig and not ins.is_transpose
                            and (si is None or not si.on_wait)):
                        if si is not None and si.on_update:
                            pending_updates.extend(si.on_update)
                        removed += 1
                        continue
                    last_sig = s
                elif getattr(ins, 'engine', None) == mybir.EngineType.PE and \
                        not isinstance(ins, mybir.InstMatmult):
                    # any other PE instruction invalidates the array state
                    last_sig = None
                if pending_updates:
                    si = ins.sync_info
                    w = list(si.on_wait) if si else []
                    u = (list(si.on_update) if si else []) + pending_updates
                    ins.sync_info = mybir.SyncInfo(on_wait=w, on_update=u)
                    pending_updates = []
                keep.append(ins)
            assert not pending_updates
            blk.instructions = keep
    return removed


def build_nc(repeat=1, with_tail=True):
    """Build the per-core Bass program. repeat>1 wraps the body in a hardware
    loop (bench mode). with_tail kept for test.py compat (no tail anymore)."""
    import concourse.bass as bass
    import concourse.mybir as mybir
    import concourse.tile as tile
    from concourse.masks import make_identity

    F32 = mybir.dt.float32
    BF16 = mybir.dt.bfloat16
    AF = mybir.ActivationFunctionType

    nc = bass.Bass()

    qt_ext = nc.declare_dram_parameter("qt", [HPC, 128, 8, S], BF16, isOutput=False)
    kt_ext = nc.declare_dram_parameter("kt", [HPC, 128, 8, S], BF16, isOutput=False)
    vt_ext = nc.declare_dram_parameter("vt", [HPC, 128, 8, S], BF16, isOutput=False)
    wz_ext = {nm: nc.declare_dram_parameter(f"w{nm}_t", [128, 8, DH], BF16, isOutput=False)
              for nm in ("q", "k", "v")}
    b_ext = {nm: nc.declare_dram_parameter(f"b{nm}2", [128, 1], F32, isOutput=False)
             for nm in ("q", "k", "v")}
    wo_ext = nc.declare_dram_parameter("wo2", [128, D], BF16, isOutput=False)
    y_ext = nc.declare_dram_parameter("y", [16, 128, D], BF16, isOutput=True)

    with tile.TileContext(nc) as tc:
        with (
            tc.tile_pool(name="consts", bufs=1) as consts,
            nc.allow_low_precision(reason="bf16 matmuls by design"),
        ):
            # ---- constants ----
            ident_f32 = consts.tile([128, 128], F32)
            make_identity(nc, ident_f32)
            ident_bf = consts.tile([128, 128], BF16)
            nc.vector.tensor_copy(ident_bf[:], ident_f32[:])
            ones128 = consts.tile([128, 1], BF16)
            nc.vector.memset(ones128, 1.0)
            onesz0 = consts.tile([1, 128], BF16)
            nc.vector.memset(onesz0[:, 0:64], 1.0)
            nc.vector.memset(onesz0[:, 64:128], 0.0)
            onesz1 = consts.tile([1, 128], BF16)
            nc.vector.memset(onesz1[:, 0:64], 0.0)
            nc.vector.memset(onesz1[:, 64:128], 1.0)

            # weight/bias DMAs are issued lazily inside the schedule so input
            # DMAs aren't delayed behind them; tiles declared here.
            biases = {}
            wz = {}
            for nm in ("q", "k", "v"):
                biases[nm] = consts.tile([128, 1], F32, tag=f"b{nm}", name=f"b{nm}")
                wz[nm] = consts.tile([128, 8, DH], BF16, tag=f"w{nm}", name=f"w{nm}")
            wo_sb = consts.tile([128, D], BF16)

            def dma_w(nm):
                # issue on Pool queue: keeps the SP queue a pure input stream
                nc.gpsimd.dma_start(out=biases[nm][:], in_=b_ext[nm][:])
                nc.gpsimd.dma_start(out=wz[nm][:], in_=wz_ext[nm][:])

            # ---- persistent per-iteration tensors ----
            kT2 = consts.tile([128, S], BF16)
            vT2 = consts.tile([128, S], BF16)
            qT2 = consts.tile([128, S], BF16)   # rows 0:64 = h0, 64:128 = h1
            va = consts.tile([128, 16, 128], BF16)  # [k, j, h-stack]
            cc_sbuf = consts.tile([128, S], BF16)
            dma_w("k")
            dma_w("q")
            dma_w("v")
            nc.gpsimd.dma_start(out=wo_sb[:], in_=wo_ext[:])

            with (
                tc.tile_pool(name="inp", bufs=7) as in_pool,
                tc.tile_pool(name="expp", bufs=EXN) as ex_pool,
                tc.tile_pool(name="denp", bufs=8) as den_pool,
                tc.tile_pool(name="accp", bufs=4) as acc_pool,
                tc.tile_pool(name="smal", bufs=4) as small_pool,
                tc.tile_pool(name="ysbp", bufs=2) as y_pool,
                tc.tile_pool(name="sc_ps", bufs=2, space="PSUM") as sc_ps,
                tc.tile_pool(name="ot_ps", bufs=2, space="PSUM") as ot_ps,
                tc.tile_pool(name="pj_ps", bufs=2, space="PSUM") as pj_ps,
            ):
                def compute_body(_iv=None):
                    # ---- input DMA: quarter tiles [128, 8, 512] on demand ----
                    ins_sb = {}

                    exts = {"k": kt_ext, "q": qt_ext, "v": vt_ext}

                    def dma_in(tname, qi, splits=1):
                        ext = exts[tname]
                        ts = []
                        for h in range(2):
                            t = in_pool.tile([128, 8, 512], BF16, tag="in",
                                             name=f"in_{tname}{h}{qi}")
                            ins_sb[(tname, h, qi)] = t
                            ts.append(t)
                        w = 512 // splits
                        for sp in range(splits):
                            for h in range(2):
                                nc.sync.dma_start(
                                    out=ts[h][:, :, sp * w:(sp + 1) * w],
                                    in_=ext[h, :, :, qi * 512 + sp * w: qi * 512 + (sp + 1) * w])

                    # ---- projection: one quarter (S-cols qi*512:+512) ----
                    def proj_q(nm, qi, splits=1):
                        ta = ins_sb.pop((nm, 0, qi))
                        tb = ins_sb.pop((nm, 1, qi))
                        wt = wz[nm]
                        ps = pj_ps.tile([128, 512], F32, tag="pp", name=f"pj{nm}{qi}")
                        w = 512 // splits
                        for sp in range(splits):
                            sub = slice(sp * w, (sp + 1) * w)
                            for c in range(8):
                                nc.tensor.matmul(ps[0:64, sub], wt[:, c, :], ta[:, c, sub],
                                                 start=(c == 0), stop=(c == 7),
                                                 skip_group_check=True)
                            for c in range(8):
                                nc.tensor.matmul(ps[64:128, sub], wt[:, c, :], tb[:, c, sub],
                                                 start=(c == 0), stop=(c == 7),
                                                 skip_group_check=True)
                        blk = slice(qi * 512, (qi + 1) * 512)
                        dst = {"k": kT2, "v": vT2, "q": qT2}[nm]
                        nc.scalar.activation(dst[:, blk], ps[:],
                                             AF.Identity, bias=biases[nm])

                    # ---- attention pieces ----
                    ex_tiles, ots, accs, dens, bcs = {}, {}, {}, {}, {}

                    def sc_group(s, j):
                        """scores + exp + den for (sqq s, key-chunk j)."""
                        s0 = s * 512
                        jsl = slice(j * 128, (j + 1) * 128)
                        sc = sc_ps.tile([128, 1024], F32, tag="sc", name=f"sc{s}_{j}")
                        nc.tensor.matmul(sc[:, 0:512], kT2[0:64, jsl],
                                         qT2[0:64, s0:s0 + 512], start=True, stop=True)
                        nc.tensor.matmul(sc[:, 512:1024], kT2[64:128, jsl],
                                         qT2[64:128, s0:s0 + 512], start=True, stop=True)
                        ex = ex_pool.tile([128, 1024], BF16, tag="ex", name=f"ex{s}_{j}")
                        nc.scalar.activation(ex[:], sc[:], AF.Exp)
                        # denominator: two interleaved accumulators on DVE
                        eng = nc.vector
                        half = j % 2
                        if j < 2:
                            den = den_pool.tile([128, 1024], BF16, tag="dn",
                                                name=f"den{s}_{half}")
                            eng.tensor_copy(den[:], ex[:])
                            dens[(s, half)] = den
                        else:
                            d = dens[(s, half)]
                            eng.tensor_add(d[:], d[:], ex[:])
                        ex_tiles[(s, j)] = ex

                    def tp_j(j):
                        """transpose v chunk j for both heads + scatter to vaz."""
                        jsl = slice(j * 128, (j + 1) * 128)
                        tp = ot_ps.tile([128, 128], BF16, tag="ot", name=f"tp{j}")
                        nc.tensor.transpose(tp[:], vT2[:, jsl], ident_bf[:])
                        nc.vector.tensor_copy(va[:, j, :], tp[:])

                    def av_q(s, jq):
                        """AV over key-quarter jq (4 j's) + fold into SBUF acc."""
                        j0, j1 = jq * 4, jq * 4 + 3
                        ot = ot_ps.tile([128, 512], F32, tag="ot", name=f"ot{s}_{jq}")
                        for j in range(j0, j1 + 1):
                            ex = ex_tiles.pop((s, j))
                            nc.tensor.matmul(ot[0:64, :], va[:, j, 0:64], ex[:, 0:512],
                                             start=(j == j0), stop=(j == j1),
                                             skip_group_check=True)
                            nc.tensor.matmul(ot[64:128, :], va[:, j, 64:128],
                                             ex[:, 512:1024],
                                             start=(j == j0), stop=(j == j1),
                                             skip_group_check=True)
                        if jq == 0:
                            acc = acc_pool.tile([128, 512], BF16, tag="acc", name=f"acc{s}")
                            nc.vector.tensor_copy(acc[:], ot[:])
                            accs[s] = acc
                        else:
                            nc.vector.tensor_add(accs[s][:], accs[s][:], ot[:])

                    def norm_a(s):
                        """den -> 1/den broadcast tile (as soon as den ready)."""
                        den_e = dens.pop((s, 0))
                        den_o = dens.pop((s, 1))
                        red = sc_ps.tile([128, 1024], F32, tag="sc", name=f"red{s}")
                        for half in (0, 1):
                            cs = slice(half * 512, (half + 1) * 512)
                            nc.tensor.matmul(red[0:1, cs], ones128[:], den_e[:, cs],
                                             start=True, stop=False,
                                             skip_group_check=True)
                            nc.tensor.matmul(red[0:1, cs], ones128[:], den_o[:, cs],
                                             start=False, stop=True,
                                             skip_group_check=True)
                        rc2 = small_pool.tile([1, 1024], BF16, tag="rc", name=f"rc{s}")
                        nc.vector.tensor_copy(rc2[:], red[0:1, :])
                        bc = sc_ps.tile([128, 1024], F32, tag="sc", name=f"bc{s}")
                        nc.tensor.matmul(bc[:, 0:512], onesz0[:], rc2[:, 0:512],
                                         start=True, stop=False, skip_group_check=True)
                        nc.tensor.matmul(bc[:, 0:512], onesz1[:], rc2[:, 512:1024],
                                         start=False, stop=True, skip_group_check=True)
                        bc_sb = small_pool.tile([128, 512], BF16, tag="bcsb", name=f"bcsb{s}")
                        nc.vector.reciprocal(bc_sb[:], bc[:, 0:512])
                        bcs[s] = bc_sb

                    def norm_b(s):
                        s0 = s * 512
                        nc.vector.tensor_mul(cc_sbuf[:, s0:s0 + 512],
                                             accs.pop(s)[:], bcs.pop(s)[:])

                    def y_chunk(s, i):
                        s0 = s * 512
                        chunk = cc_sbuf[:, s0 + i * 128: s0 + (i + 1) * 128]
                        ya = pj_ps.tile([128, 512], F32, tag="pp", name=f"ya{s}{i}")
                        yb = pj_ps.tile([128, 512], F32, tag="pp", name=f"yb{s}{i}")
                        nc.tensor.matmul(ya[:], chunk, wo_sb[:, 0:512],
                                         start=True, stop=True)
                        nc.tensor.matmul(yb[:], chunk, wo_sb[:, 512:1024],
                                         start=True, stop=True)
                        ysb = y_pool.tile([128, 1024], BF16, tag="ysb", name=f"y{s}{i}")
                        nc.scalar.activation(ysb[:, 0:512], ya[:], AF.Copy)
                        nc.vector.tensor_copy(ysb[:, 512:1024], yb[:])
                        # SWDGE (gpsimd) DMAs fail walrus codegen inside a
                        # hardware loop; use them only in the single-shot build
                        yeng = nc.gpsimd if repeat == 1 else nc.sync
                        yeng.dma_start(out=y_ext[s * 4 + i, :, :], in_=ysb[:])

                    # ================= SCHEDULE =================
                    # The SP queue is a pure input stream in deadline order
                    # (consts went out on the Pool queue before the body).
                    dma_in("k", 0, splits=2)
                    dma_in("q", 0, splits=2)
                    dma_in("k", 1)
                    dma_in("q", 1)
                    dma_in("v", 0)
                    dma_in("k", 2)
                    dma_in("k", 3)
                    dma_in("v", 1)
                    dma_in("q", 2)
                    dma_in("v", 2)
                    dma_in("v", 3)
                    dma_in("q", 3)

                    proj_q("k", 0, splits=2)    # first k bytes land ~@4.5
                    proj_q("q", 0, splits=2)
                    for j in range(0, 4):       # kT j0:3, qz s0 ready ~@15
                        sc_group(0, j)
                    proj_q("k", 1)              # k q1 ~@21
                    for j in range(4, 8):
                        sc_group(0, j)
                    proj_q("q", 1)              # q q1 ~@27
                    for j in range(0, 8):
                        sc_group(1, j)
                    proj_q("v", 0)              # v q0 ~@35
                    for j in range(0, 4):
                        tp_j(j)
                    av_q(0, 0)
                    av_q(1, 0)
                    proj_q("k", 2)              # k q2 ~@41
                    for j in range(8, 12):
                        sc_group(0, j)
                        sc_group(1, j)
                    proj_q("k", 3)              # k q3 ~@47
                    for j in range(12, 16):
                        sc_group(0, j)
                        sc_group(1, j)
                    norm_a(0)
                    norm_a(1)
                    proj_q("v", 1)              # v q1 ~@53
                    for j in range(4, 8):
                        tp_j(j)
                    av_q(0, 1)
                    av_q(1, 1)
                    proj_q("q", 2)              # q q2 ~@59
                    for j in range(0, 8):       # s2 with JIT AV
                        sc_group(2, j)
                        if j == 5:
                            av_q(2, 0)
                    proj_q("v", 2)              # v q2 ~@65
                    for j in range(8, 12):
                        tp_j(j)
                    av_q(0, 2)
                    av_q(1, 2)
                    av_q(2, 1)
                    for j in range(8, 16):      # s2 tail scores, JIT AV
                        sc_group(2, j)
                        if j == 13:
                            av_q(2, 2)
                    norm_a(2)
                    proj_q("q", 3)              # q q3 ~@76
                    proj_q("v", 3)              # v q3 ~@70
                    for j in range(12, 16):
                        tp_j(j)
                    av_q(0, 3)
                    norm_b(0)
                    av_q(1, 3)
                    norm_b(1)
                    av_q(2, 3)
                    norm_b(2)
                    # s0..s2 output GEMMs interleave with s3's attention tail
                    yq = [(s, i) for i in range(4) for s in range(3)]
                    for j in range(0, 16):      # s3 JIT: sc -> AV lag
                        sc_group(3, j)
                        if j == 5:
                            av_q(3, 0)
                        elif j == 9:
                            av_q(3, 1)
                        elif j == 13:
                            av_q(3, 2)
                        elif yq:
                            y_chunk(*yq.pop(0))
                    norm_a(3)
                    av_q(3, 3)
                    while yq:
                        y_chunk(*yq.pop(0))
                    norm_b(3)
                    for i in range(4):
                        y_chunk(3, i)


# revision 24
# speedup vs baseline: 1.6422x; 1.0817x over previous
"""Trainium2 Bass kernel for nn_MultiHeadAttention (B=16 heads, S=2048, D=1024, DH=64).

Sharding: 2 heads per core across 8 cores (head-parallel), NO collectives:
each core computes its 2 heads' slice of the final W_O GEMM (contract over
the 128 local head-dims) into a full-shape [S, D] bf16 partial; the host
sums the 8 partials and adds bo. Data that must cross cores is identical
either way; this removes collective latency and the cc readback.

Per-core program (all bf16 matmuls, fp32 PSUM):
  - projections: c-outer accumulation with zero-padded stationaries
    wz0=[W|0], wz1=[0|W] so both heads' halves accumulate into one full
    [128,512] PSUM tile (junk rows are exact zeros) and consecutive
    matmuls share identical ldweights (deduped post-pass).
  - scores: stationary kT2[:,j*128:+128] [128,128] (both heads stacked),
    moving = zero-padded qz0/qz1 so the cross-head contract terms vanish;
    one ldweights per (sqq,j) pair after dedup. exp on ACT engine
    ([128,1024] per group), denominator accumulated on DVE.
  - AV: stationary vaz0=[va_h0|0], vaz1=[0|va_h1] accumulate both heads
    into one [128,512] PSUM tile; j0:7 partial stashed to SBUF (bf16) so
    only 2 PSUM ot-tiles are ever live.
  - y: per sqq, 4 stationary cc-chunks [128,128] x moving Wo2 [128,1024]
    -> y s-chunk [128,1024] fp32 -> bf16 -> DMA out. 16 chunks total.
  - startup: S-half input DMAs (k,q,v interleaved by deadline) feed a
    phase schedule that keeps PE busy from ~6us on.
"""
import sys, os
sys.path.insert(0, '/opt/trn_rl_repo')
import numpy as np

B = 16        # total heads
S = 2048
D = 1024
DH = 64
N_CORES = 8
HPC = B // N_CORES          # heads per core = 2
EXN = int(os.environ.get("KEXN", "24"))   # ex-tile pool depth

_runner = None


def _split_excess_waits(nc, mybir):
    """walrus in this env supports only ONE sync-wait command per instruction;
    hoist extra waits onto preceding single-wait NOPs on the same engine."""
    for f in nc.m.functions:
        for blk in f.blocks:
            new_list = []
            changed = False
            for ins in blk.instructions:
                si = ins.sync_info
                if si is not None and si.on_wait and len(si.on_wait) > 1:
                    waits = list(si.on_wait)
                    extra, keep = waits[:-1], waits[-1:]
                    for ci, w in enumerate(extra):
                        nop = mybir.InstNoOp(name=f"{ins.name}_wsplit_{ci}", ins=[], outs=[])
                        nop.engine = ins.engine
                        nop.sync_info = mybir.SyncInfo(on_wait=[w], on_update=[])
                        new_list.append(nop)
                    ins.sync_info = mybir.SyncInfo(on_wait=keep, on_update=list(si.on_update))
                    changed = True
                new_list.append(ins)
            if changed:
                blk.instructions = new_list


def _dedup_ldweights(nc, mybir):
    """Remove an InstLdweights that re-loads exactly what the previous
    InstLdweights on PE already loaded (same AP/tile/flags). The legalizer
    emits one ldweights per matmult even when consecutive matmuls share the
    stationary; the PE array retains it, so the duplicate is a pure re-load.
    Only drop duplicates that carry no waits; sem updates are moved to the
    next instruction."""

    def sig(ins):
        pap = ins.ins[0]
        return (pap.offset, tuple(tuple(x) for x in pap.ap), str(pap.dtype),
                tuple(ins.tile_position or ()), tuple(ins.tile_size or ()),
                bool(ins.is_transpose), ins.perf_mode)

    removed = 0
    for f in nc.m.functions:
        for blk in f.blocks:
            last_sig = None
            keep = []
            pending_updates = []
            for ins in blk.instructions:
                if isinstance(ins, mybir.InstLdweights):
                    s = sig(ins)
                    si = ins.sync_info
                    if (s == last_sig and not ins.is_transpose
                            and (si is None or not si.on_wait)):
                        if si is not None and si.on_update:
                            pending_updates.extend(si.on_update)
                        removed += 1
                        continue
                    last_sig = s
                elif getattr(ins, 'engine', None) == mybir.EngineType.PE and \
                        not isinstance(ins, mybir.InstMatmult):
                    # any other PE instruction invalidates the array state
                    last_sig = None
                if pending_updates:
                    si = ins.sync_info
                    w = list(si.on_wait) if si else []
                    u = (list(si.on_update) if si else []) + pending_updates
                    ins.sync_info = mybir.SyncInfo(on_wait=w, on_update=u)
                    pending_updates = []
                keep.append(ins)
            assert not pending_updates
            blk.instructions = keep
    return removed


def build_nc(repeat=1, with_tail=True):
    """Build the per-core Bass program. repeat>1 wraps the body in a hardware
    loop (bench mode). with_tail kept for test.py compat (no tail anymore)."""
    import concourse.bass as bass
    import concourse.mybir as mybir
    import concourse.tile as tile
    from concourse.masks import make_identity

    F32 = mybir.dt.float32
    BF16 = mybir.dt.bfloat16
    AF = mybir.ActivationFunctionType

    nc = bass.Bass()

    qt_ext = nc.declare_dram_parameter("qt", [HPC, 128, 8, S], BF16, isOutput=False)
    kt_ext = nc.declare_dram_parameter("kt", [HPC, 128, 8, S], BF16, isOutput=False)
    vt_ext = nc.declare_dram_parameter("vt", [HPC, 128, 8, S], BF16, isOutput=False)
    wz_ext = {nm: nc.declare_dram_parameter(f"w{nm}_t", [128, 8, DH], BF16, isOutput=False)
              for nm in ("q", "k", "v")}
    b_ext = {nm: nc.declare_dram_parameter(f"b{nm}2", [128, 1], F32, isOutput=False)
             for nm in ("q", "k", "v")}
    wo_ext = nc.declare_dram_parameter("wo2", [128, D], BF16, isOutput=False)
    y_ext = nc.declare_dram_parameter("y", [16, 128, D], BF16, isOutput=True)

    with tile.TileContext(nc) as tc:
        with (
            tc.tile_pool(name="consts", bufs=1) as consts,
            nc.allow_low_precision(reason="bf16 matmuls by design"),
        ):
            # ---- constants ----
            ident_f32 = consts.tile([128, 128], F32)
            make_identity(nc, ident_f32)
            ident_bf = consts.tile([128, 128], BF16)
            nc.vector.tensor_copy(ident_bf[:], ident_f32[:])
            ones128 = consts.tile([128, 1], BF16)
            nc.vector.memset(ones128, 1.0)
            onesz0 = consts.tile([1, 128], BF16)
            nc.vector.memset(onesz0[:, 0:64], 1.0)
            nc.vector.memset(onesz0[:, 64:128], 0.0)
            onesz1 = consts.tile([1, 128], BF16)
            nc.vector.memset(onesz1[:, 0:64], 0.0)
            nc.vector.memset(onesz1[:, 64:128], 1.0)

            # weight/bias DMAs are issued lazily inside the schedule so input
            # DMAs aren't delayed behind them; tiles declared here.
            biases = {}
            wz = {}
            for nm in ("q", "k", "v"):
                biases[nm] = consts.tile([128, 1], F32, tag=f"b{nm}", name=f"b{nm}")
                wz[nm] = consts.tile([128, 8, DH], BF16, tag=f"w{nm}", name=f"w{nm}")
            wo_sb = consts.tile([128, D], BF16)

            def dma_w(nm):
                # issue on Pool queue: keeps the SP queue a pure input stream
                nc.gpsimd.dma_start(out=biases[nm][:], in_=b_ext[nm][:])
                nc.gpsimd.dma_start(out=wz[nm][:], in_=wz_ext[nm][:])

            # ---- persistent per-iteration tensors ----
            kT2 = consts.tile([128, S], BF16)
            vT2 = consts.tile([128, S], BF16)
            qT2 = consts.tile([128, S], BF16)   # rows 0:64 = h0, 64:128 = h1
            va = consts.tile([128, 16, 128], BF16)  # [k, j, h-stack]
            cc_sbuf = consts.tile([128, S], BF16)
            dma_w("k")
            dma_w("q")
            dma_w("v")
            nc.gpsimd.dma_start(out=wo_sb[:], in_=wo_ext[:])

            with (
                tc.tile_pool(name="inp", bufs=7) as in_pool,
                tc.tile_pool(name="expp", bufs=EXN) as ex_pool,
                tc.tile_pool(name="denp", bufs=16) as den_pool,
                tc.tile_pool(name="accp", bufs=4) as acc_pool,
                tc.tile_pool(name="smal", bufs=4) as small_pool,
                tc.tile_pool(name="ysbp", bufs=2) as y_pool,
                tc.tile_pool(name="sc_ps", bufs=2, space="PSUM") as sc_ps,
                tc.tile_pool(name="ot_ps", bufs=2, space="PSUM") as ot_ps,
                tc.tile_pool(name="pj_ps", bufs=2, space="PSUM") as pj_ps,
            ):
                def compute_body(_iv=None):
                    # ---- input DMA: quarter tiles [128, 8, 512] on demand ----
                    ins_sb = {}

                    exts = {"k": kt_ext, "q": qt_ext, "v": vt_ext}

                    def dma_in(tname, qi, splits=1):
                        ext = exts[tname]
                        ts = []
                        for h in range(2):
                            t = in_pool.tile([128, 8, 512], BF16, tag="in",
                                             name=f"in_{tname}{h}{qi}")
                            ins_sb[(tname, h, qi)] = t
                            ts.append(t)
                        w = 512 // splits
                        for sp in range(splits):
                            for h in range(2):
                                nc.sync.dma_start(
                                    out=ts[h][:, :, sp * w:(sp + 1) * w],
                                    in_=ext[h, :, :, qi * 512 + sp * w: qi * 512 + (sp + 1) * w])

                    # ---- projection: one quarter (S-cols qi*512:+512) ----
                    def proj_q(nm, qi, splits=1):
                        ta = ins_sb.pop((nm, 0, qi))
                        tb = ins_sb.pop((nm, 1, qi))
                        wt = wz[nm]
                        ps = pj_ps.tile([128, 512], F32, tag="pp", name=f"pj{nm}{qi}")
                        w = 512 // splits
                        for sp in range(splits):
                            sub = slice(sp * w, (sp + 1) * w)
                            for c in range(8):
                                nc.tensor.matmul(ps[0:64, sub], wt[:, c, :], ta[:, c, sub],
                                                 start=(c == 0), stop=(c == 7),
                                                 skip_group_check=True)
                            for c in range(8):
                                nc.tensor.matmul(ps[64:128, sub], wt[:, c, :], tb[:, c, sub],
                                                 start=(c == 0), stop=(c == 7),
                                                 skip_group_check=True)
                        blk = slice(qi * 512, (qi + 1) * 512)
                        dst = {"k": kT2, "v": vT2, "q": qT2}[nm]
                        nc.scalar.activation(dst[:, blk], ps[:],
                                             AF.Identity, bias=biases[nm])

                    # ---- attention pieces ----
                    ex_tiles, ots, accs, dens, bcs = {}, {}, {}, {}, {}

                    def sc_group(s, j):
                        """scores + exp + den for (sqq s, key-chunk j)."""
                        s0 = s * 512
                        jsl = slice(j * 128, (j + 1) * 128)
                        sc = sc_ps.tile([128, 1024], F32, tag="sc", name=f"sc{s}_{j}")
                        nc.tensor.matmul(sc[:, 0:512], kT2[0:64, jsl],
                                         qT2[0:64, s0:s0 + 512], start=True, stop=True)
                        nc.tensor.matmul(sc[:, 512:1024], kT2[64:128, jsl],
                                         qT2[64:128, s0:s0 + 512], start=True, stop=True)
                        ex = ex_pool.tile([128, 1024], BF16, tag="ex", name=f"ex{s}_{j}")
                        nc.scalar.activation(ex[:], sc[:], AF.Exp)
                        # denominator: four interleaved accumulators on DVE
                        eng = nc.vector
                        half = j % 4
                        if j < 4:
                            den = den_pool.tile([128, 1024], BF16, tag="dn",
                                                name=f"den{s}_{half}")
                            eng.tensor_copy(den[:], ex[:])
                            dens[(s, half)] = den
                        else:
                            d = dens[(s, half)]
                            eng.tensor_add(d[:], d[:], ex[:])
                        ex_tiles[(s, j)] = ex

                    def tp_j(j):
                        """transpose v chunk j for both heads + scatter to vaz."""
                        jsl = slice(j * 128, (j + 1) * 128)
                        tp = ot_ps.tile([128, 128], BF16, tag="ot", name=f"tp{j}")
                        nc.tensor.transpose(tp[:], vT2[:, jsl], ident_bf[:])
                        nc.vector.tensor_copy(va[:, j, :], tp[:])

                    def av_q(s, jq):
                        """AV over key-quarter jq (4 j's) + fold into SBUF acc."""
                        j0, j1 = jq * 4, jq * 4 + 3
                        ot = ot_ps.tile([128, 512], F32, tag="ot", name=f"ot{s}_{jq}")
                        for j in range(j0, j1 + 1):
                            ex = ex_tiles.pop((s, j))
                            nc.tensor.matmul(ot[0:64, :], va[:, j, 0:64], ex[:, 0:512],
                                             start=(j == j0), stop=(j == j1),
                                             skip_group_check=True)
                            nc.tensor.matmul(ot[64:128, :], va[:, j, 64:128],
                                             ex[:, 512:1024],
                                             start=(j == j0), stop=(j == j1),
                                             skip_group_check=True)
                        if jq == 0:
                            acc = acc_pool.tile([128, 512], BF16, tag="acc", name=f"acc{s}")
                            nc.vector.tensor_copy(acc[:], ot[:])
                            accs[s] = acc
                        else:
                            nc.vector.tensor_add(accs[s][:], accs[s][:], ot[:])

                    def norm_a(s):
                        """den -> 1/den broadcast tile (as soon as den ready)."""
                        dts = [dens.pop((s, h)) for h in range(4)]
                        red = sc_ps.tile([128, 1024], F32, tag="sc", name=f"red{s}")
                        for half in (0, 1):
                            cs = slice(half * 512, (half + 1) * 512)
                            for hi, dt in enumerate(dts):
                                nc.tensor.matmul(red[0:1, cs], ones128[:], dt[:, cs],
                                                 start=(hi == 0), stop=(hi == 3),
                                                 skip_group_check=True)
                        rc2 = small_pool.tile([1, 1024], BF16, tag="rc", name=f"rc{s}")
                        nc.vector.tensor_copy(rc2[:], red[0:1, :])
                        bc = sc_ps.tile([128, 1024], F32, tag="sc", name=f"bc{s}")
                        nc.tensor.matmul(bc[:, 0:512], onesz0[:], rc2[:, 0:512],
                                         start=True, stop=False, skip_group_check=True)
                        nc.tensor.matmul(bc[:, 0:512], onesz1[:], rc2[:, 512:1024],
                                         start=False, stop=True, skip_group_check=True)
                        bc_sb = small_pool.tile([128, 512], BF16, tag="bcsb", name=f"bcsb{s}")
                        nc.vector.reciprocal(bc_sb[:], bc[:, 0:512])
                        bcs[s] = bc_sb

                    def norm_b(s):
                        s0 = s * 512
                        nc.vector.tensor_mul(cc_sbuf[:, s0:s0 + 512],
                                             accs.pop(s)[:], bcs.pop(s)[:])

                    def y_chunk(s, i):
                        s0 = s * 512
                        chunk = cc_sbuf[:, s0 + i * 128: s0 + (i + 1) * 128]
                        ya = pj_ps.tile([128, 512], F32, tag="pp", name=f"ya{s}{i}")
                        yb = pj_ps.tile([128, 512], F32, tag="pp", name=f"yb{s}{i}")
                        nc.tensor.matmul(ya[:], chunk, wo_sb[:, 0:512],
                                         start=True, stop=True)
                        nc.tensor.matmul(yb[:], chunk, wo_sb[:, 512:1024],
                                         start=True, stop=True)
                        ysb = y_pool.tile([128, 1024], BF16, tag="ysb", name=f"y{s}{i}")
                        nc.scalar.activation(ysb[:, 0:512], ya[:], AF.Copy)
                        nc.vector.tensor_copy(ysb[:, 512:1024], yb[:])
                        # SWDGE (gpsimd) DMAs fail walrus codegen inside a
                        # hardware loop; use them only in the single-shot build
                        yeng = nc.gpsimd if repeat == 1 else nc.sync
                        yeng.dma_start(out=y_ext[s * 4 + i, :, :], in_=ysb[:])

                    # ================= SCHEDULE =================
                    # The SP queue is a pure input stream in deadline order
                    # (consts went out on the Pool queue before the body).
                    dma_in("k", 0, splits=2)
                    dma_in("q", 0, splits=2)
                    dma_in("k", 1)
                    dma_in("q", 1)
                    dma_in("v", 0)
                    dma_in("k", 2)
                    dma_in("k", 3)
                    dma_in("v", 1)
                    dma_in("q", 2)
                    dma_in("v", 2)
                    dma_in("v", 3)
                    dma_in("q", 3)

                    proj_q("k", 0, splits=2)    # first k bytes land ~@4.5
                    proj_q("q", 0, splits=2)
                    for j in range(0, 4):       # kT j0:3, qz s0 ready ~@15
                        sc_group(0, j)
                    proj_q("k", 1)              # k q1 ~@21
                    for j in range(4, 8):
                        sc_group(0, j)
                    proj_q("q", 1)              # q q1 ~@27
                    for j in range(0, 8):
                        sc_group(1, j)
                    proj_q("v", 0)              # v q0 ~@35
                    for j in range(0, 4):
                        tp_j(j)
                    av_q(0, 0)
                    av_q(1, 0)
                    proj_q("k", 2)              # k q2 ~@41
                    for j in range(8, 12):
                        sc_group(0, j)
                        sc_group(1, j)
                    proj_q("k", 3)              # k q3 ~@47
                    for j in range(12, 16):
                        sc_group(0, j)
                        sc_group(1, j)
                    norm_a(0)
                    norm_a(1)
                    proj_q("v", 1)              # v q1 ~@53
                    for j in range(4, 8):
                        tp_j(j)
                    av_q(0, 1)
                    av_q(1, 1)
                    proj_q("q", 2)              # q q2 ~@59
                    for j in range(0, 8):       # s2 with JIT AV
                        sc_group(2, j)
                        if j == 5:
                            av_q(2, 0)
                    proj_q("v", 2)              # v q2 ~@65
                    for j in range(8, 12):
                        tp_j(j)
                    av_q(0, 2)
                    av_q(1, 2)
                    av_q(2, 1)
                    for j in range(8, 16):      # s2 tail scores, JIT AV
                        sc_group(2, j)
                        if j == 13:
                            av_q(2, 2)
                    norm_a(2)
                    proj_q("q", 3)              # q q3 ~@76
                    proj_q("v", 3)              # v q3 ~@70
                    for j in range(12, 16):
                        tp_j(j)
                    av_q(0, 3)
                    norm_b(0)
                    av_q(1, 3)
                    norm_b(1)
                    av_q(2, 3)
                    norm_b(2)
                    # s0..s2 output GEMMs interleave with s3's attention tail
                    yq = [(s, i) for i in range(4) for s in range(3)]
                    for j in range(0, 16):      # s3 JIT: sc -> AV lag
                        sc_group(3, j)
                        if j == 5:
                            av_q(3, 0)
                        elif j == 9:
                            av_q(3, 1)
                        elif j == 13:
                            av_q(3, 2)
                        elif yq:
                            y_chunk(*yq.pop(0))
                    norm_a(3)
                    av_q(3, 3)
                    while yq:
                        y_chunk(*yq.pop(0))
                    norm_b(3)
                    for i in range(4):
                        y_chunk(3, i)
